# revision 53
# baseline (speedup 1.0000x reference)
"""Contrastive queue loss kernel for 8 Trainium2 NeuronCores.

Reference computation (all fp32):
    pos[j,b]    = V[j,b,:] . L[b,:] / T                  (J=2, B=256, F=128)
    qlog[j,b,q] = V[j,b,:] . queue[q,:] / T              (Q=65536)
    denom[j,b]  = log( sum_i exp(pos[j,i]) + sum_q exp(qlog[j,b,q]) )
    loss        = -sum_{j,b} (pos[j,b] - denom[j,b]) / B

Sharding: queue split along Q across 8 cores (8192 rows each); V replicated.
Each core emits per-(jb, group) partials of sum_q exp(10*logit[jb, q]).
pos (65K MACs) and the final logsumexp combine run on the host in float64 —
the device does the 33.5M-exp / 4.3-GFLOP queue part only.

Structural choices (informed by NTFF traces):
  * sum_q exp is invariant to q permutation -> the queue shard is DMAed
    CONTIGUOUSLY (0.5-4KB per partition line, near-peak HBM rate), and the
    PE-transposed blocks come out q-permuted, which is fine.
  * V arrives pre-transposed from the host (V2T [f, jb]) so the device does
    no V transpose, just a tiny f32->bf16 cast.
  * The scalar engine's exp (1 elem/lane/cycle, ~1.5us per 1536-wide op)
    is the roofline: ~33.5us of ACT time per core.  Offloading part of the
    exp to the Vector engine (Schraudolph bit-trick) was measured to be a
    wash: concurrent DVE reads of PSUM slow every ACTIVATE by ~20%
    (PSUM-port contention), and the chip's activity throttler
    (util limit 0.5) claws back the rest.  Pure-ACT exp won every A/B.
  * Queue chunks ramp 128->1024 rows so the first ACT op issues right
    after the first 128 queue rows land.  Chunk DMAs + bf16 casts are
    emitted eagerly up front (DMA streams at its own pace); the PE
    transposes + SBUF copies are emitted lazily, just before the group
    that consumes them, so a transpose whose data hasn't arrived never
    blocks ready matmuls in the PE FIFO.
  * Keeping DMA burst rate moderate matters: larger early chunks measured
    ~10us SLOWER end-to-end because the higher DMA+ACT+PE activity trips
    the hardware utilization throttle (ACTIVATE durs inflate 1.2x).

Per-core dataflow:
  DMA queue chunk (contiguous fp32) -> DVE cast to bf16
  -> PE 128x128 identity-matmul transposes into PSUM -> DVE copy to SBUF
  -> PE matmul against persistent V2T (bf16) -> logits in PSUM (fp32),
     one 512-col PSUM bank per matmul
  -> ACT exp(10x) in place with fused accumulation (accum_out)
  -> DMA the [128, NT*NG] partial-sum tile out; host sums group partials.
"""

import numpy as np

J, B, F, Q = 2, 256, 128, 65536
NCORES = 8
QC = Q // NCORES          # 8192 queue rows per core
JB = J * B                # 512
INV_T = 10.0
NT = JB // 128            # 4 jb tiles of 128
TEMPERATURE = 0.1

# (row_start, nrows) queue chunks, contiguous per partition; small first
# chunks shorten the pipeline ramp to the first ACT op.
CHUNKS = [(0, 128), (128, 384), (512, 512)] + \
         [(r, 1024) for r in range(1024, QC, 1024)]
assert sum(nr for _, nr in CHUNKS) == QC

# Per-jb-tile scalar-engine exp groups over the 8192 q columns; <= 1536
# cols each (3 PSUM banks; pool bufs=2 + transpose-staging bank = 7 of 8).
# Small leading groups start the ACT engine while the DMA is still ramping.
GROUPS = [
    ('a', 0, 128),
    ('a', 128, 1024),
    ('a', 1024, 2560),
    ('a', 2560, 4096),
    ('a', 4096, 5632),
    ('a', 5632, 7168),
    ('a', 7168, 8192),
]
assert all(c1 - c0 <= 1536 for _, c0, c1 in GROUPS)
assert GROUPS[0][1] == 0 and GROUPS[-1][2] == QC
NG = len(GROUPS)

# Flat (group, tile) op order: ramp groups first, then column order.
SCHED_OPS = [(g, t) for g in range(NG) for t in range(NT)]

_STATE = {}


def _build():
    import concourse.tile as tile
    from concourse import bacc, masks, mybir

    f32 = mybir.dt.float32
    bf16 = mybir.dt.bfloat16
    nc = bacc.Bacc("TRN2", target_bir_lowering=False, debug=False,
                   num_devices=None, enable_partition_id=False)

    vt_d = nc.dram_tensor("V2T", (128, JB), f32, kind="ExternalInput")
    q_d = nc.dram_tensor("queue", (QC, F), f32, kind="ExternalInput")
    # out[p, t*NG + g] = group-g partial of sum_q exp(10 * logit[jb, q]),
    # jb = t*128 + p; the host sums the NG group partials per jb.
    out_d = nc.dram_tensor("out", (128, NT * NG), f32, kind="ExternalOutput")

    # chunk column offsets (transposed q-cols land in chunk order)
    coff = []
    acc_cols = 0
    for _, nr in CHUNKS:
        coff.append(acc_cols)
        acc_cols += nr

    with tile.TileContext(nc) as tc:
        with (
            tc.tile_pool(name="const", bufs=1) as const_pool,
            tc.tile_pool(name="vl", bufs=1) as vl_pool,
            tc.tile_pool(name="qt", bufs=12) as qt_pool,
            tc.tile_pool(name="qtb", bufs=12) as qtb_pool,
            tc.tile_pool(name="qts", bufs=12) as qts_pool,
            tc.tile_pool(name="res", bufs=1) as res_pool,
            tc.tile_pool(name="pslog", bufs=2, space="PSUM") as pslog_pool,
            tc.tile_pool(name="pst", bufs=1, space="PSUM") as pst_pool,
        ):
            identb = const_pool.tile([128, 128], bf16, tag="identb")
            masks.make_identity(nc, identb[:])

            # DGE wake primer: one 512B single-descriptor transfer issued
            # first, so the DMA engine's first-transfer wake-up latency
            # (~2us) is paid on a throwaway instead of delaying chunk 0.
            primer = const_pool.tile([1, F], f32, tag="primer")
            nc.sync.dma_start(primer[:], q_d.ap()[0:1, :])

            # V2T [f=128, jb=512] comes pre-transposed from the host.
            vt_f = vl_pool.tile([128, JB], f32)
            nc.sync.dma_start(vt_f[:], vt_d.ap())
            v2tb = vl_pool.tile([128, JB], bf16)
            nc.vector.tensor_copy(v2tb[:], vt_f[:])

            # ---- queue chunk pipeline, split in two phases:
            # (a) DMA + bf16 cast, emitted EAGERLY for every chunk up front —
            #     the DMA engines stream at their own pace and the DVE casts
            #     drain behind them;
            # (b) PE transpose + copy-to-SBUF, emitted LAZILY just before the
            #     consuming exp group — a transpose queued in the PE FIFO
            #     before its data arrived would block later, ready matmuls.
            qtbs = []                  # phase-(a) results: bf16 natural tiles
            for r0, nr in CHUNKS:
                sr = nr // 128                         # rows per partition
                qt = qt_pool.tile([128, nr], f32, tag="qt")
                nc.sync.dma_start(
                    qt[:].rearrange("p (s f) -> p s f", f=F),
                    q_d.ap()[r0:r0 + nr, :].rearrange(
                        "(p s) f -> p s f", s=sr))
                qtb = qtb_pool.tile([128, nr], bf16, tag="qtb")
                nc.vector.tensor_copy(qtb[:], qt[:])
                qtbs.append(qtb)

            segs = []                  # (qts_tile, global_col0, n)
            _next_chunk = [0]

            def emit_chunk():
                ci = _next_chunk[0]
                _next_chunk[0] += 1
                _, nr = CHUNKS[ci]
                qtb = qtbs[ci]
                pt = pst_pool.tile([128, nr], bf16, tag="pst")
                for s in range(nr // 128):
                    nc.tensor.transpose(
                        pt[:, s * 128:(s + 1) * 128],
                        qtb[:, s * 128:(s + 1) * 128], identb[:])
                qts = qts_pool.tile([128, nr], bf16, tag="qts")
                nc.vector.tensor_copy(qts[:], pt[:])
                segs.append((qts, coff[ci], nr))

            def ensure_cols(c):
                """Emit transpose pipelines until columns [0, c) are covered."""
                while _next_chunk[0] < len(CHUNKS) and \
                        (not segs or segs[-1][1] + segs[-1][2] < c):
                    emit_chunk()

            PREFETCH = 0               # transposes are just-in-time; casts are eager

            def emit_matmuls(lg, t, c0, c1):
                """Matmuls filling lg[:, 0:c1-c0] with logits for jb tile t,
                global q columns [c0, c1).  Each matmul stays within one
                PSUM bank (512 f32) of lg."""
                for qts, g0, n in segs:
                    o0, o1 = max(c0, g0), min(c1, g0 + n)
                    a = o0
                    while a < o1:
                        b = min(o1, a + 512 - (a - c0) % 512)
                        nc.tensor.matmul(
                            lg[:, a - c0:b - c0],
                            lhsT=v2tb[:, t * 128:(t + 1) * 128],
                            rhs=qts[:, a - g0:b - g0], start=True, stop=True)
                        a = b

            # ---- logits + fused exp/accumulate ----
            # acc[p, t*NG + g] = partial sum for jb tile t, group g
            acc = res_pool.tile([128, NT * NG], f32)
            for gi, t in SCHED_OPS:
                _, c0, c1 = GROUPS[gi]
                ensure_cols(min(c1 + PREFETCH, QC))
                w = c1 - c0
                col = t * NG + gi
                lg = pslog_pool.tile([128, w], f32, tag="pslog")
                emit_matmuls(lg, t, c0, c1)
                nc.scalar.activation(
                    lg[:], lg[:], mybir.ActivationFunctionType.Exp,
                    scale=INV_T, accum_out=acc[:, col:col + 1])

            # ---- finalize: DMA the raw group partials; host sums them ----
            nc.sync.dma_start(out_d.ap(), acc[:])

    nc.compile()
    return nc


def _run(in_maps, trace=False, **kwargs):
    from concourse.bass_utils import run_bass_kernel_spmd
    if "nc" not in _STATE:
        _STATE["nc"] = _build()
    return run_bass_kernel_spmd(_STATE["nc"], in_maps, list(range(NCORES)),
                                trace=trace, **kwargs)


def _make_in_maps(V, L, queue):
    V2T = np.ascontiguousarray(
        np.asarray(V, dtype=np.float32).reshape(JB, F).T)
    qn = np.asarray(queue, dtype=np.float32).reshape(NCORES, QC, F)
    return [{"V2T": V2T, "queue": np.ascontiguousarray(qn[i])}
            for i in range(NCORES)]


def _combine(V, L, outs):
    """outs: list of (128, NT*NG) partial arrays, one per core -> loss."""
    qsum = np.zeros(JB, dtype=np.float64)
    for o in outs:
        per_jb = o.astype(np.float64).reshape(128, NT, NG).sum(-1)
        qsum += per_jb.T.reshape(JB)                 # jb = t*128 + p
    V2 = np.asarray(V, dtype=np.float64).reshape(JB, F)
    Ln = np.asarray(L, dtype=np.float64)
    pos = (V2.reshape(J, B, F) * Ln[None]).sum(-1).reshape(JB) / TEMPERATURE
    batch_sum = np.exp(pos).reshape(J, B).sum(axis=1)  # sum_i exp(pos[j,i])
    denom = np.log(np.repeat(batch_sum, B) + qsum)
    loss = -(pos.sum() - denom.sum()) / B
    return np.array(loss, dtype=np.float32)


def kernel(V, L, queue):
    res = _run(_make_in_maps(V, L, queue))
    return _combine(V, L, [res.results[i]["out"] for i in range(NCORES)])


# revision 54
# speedup vs baseline: 1.0243x; 1.0243x over previous
"""Contrastive queue loss kernel for 8 Trainium2 NeuronCores.

Reference computation (all fp32):
    pos[j,b]    = V[j,b,:] . L[b,:] / T                  (J=2, B=256, F=128)
    qlog[j,b,q] = V[j,b,:] . queue[q,:] / T              (Q=65536)
    denom[j,b]  = log( sum_i exp(pos[j,i]) + sum_q exp(qlog[j,b,q]) )
    loss        = -sum_{j,b} (pos[j,b] - denom[j,b]) / B

Sharding: queue split along Q across 8 cores (8192 rows each); V replicated.
Each core emits per-(jb, group) partials of sum_q exp(10*logit[jb, q]).
pos (65K MACs) and the final logsumexp combine run on the host in float64 —
the device does the 33.5M-exp / 4.3-GFLOP queue part only.

Structural choices (informed by NTFF traces):
  * sum_q exp is invariant to q permutation -> the queue shard is DMAed
    CONTIGUOUSLY (0.5-4KB per partition line, near-peak HBM rate), and the
    PE-transposed blocks come out q-permuted, which is fine.
  * V arrives pre-transposed from the host (V2T [f, jb]) so the device does
    no V transpose, just a tiny f32->bf16 cast.
  * The scalar engine's exp (1 elem/lane/cycle, ~1.5us per 1536-wide op)
    is the roofline: ~33.5us of ACT time per core.  Offloading part of the
    exp to the Vector engine (Schraudolph bit-trick) was measured to be a
    wash: concurrent DVE reads of PSUM slow every ACTIVATE by ~20%
    (PSUM-port contention), and the chip's activity throttler
    (util limit 0.5) claws back the rest.  Pure-ACT exp won every A/B.
  * Queue chunks ramp 128->1024 rows so the first ACT op issues right
    after the first 128 queue rows land.  Chunk DMAs + bf16 casts are
    emitted eagerly up front (DMA streams at its own pace); the PE
    transposes + SBUF copies are emitted lazily, just before the group
    that consumes them, so a transpose whose data hasn't arrived never
    blocks ready matmuls in the PE FIFO.
  * Keeping DMA burst rate moderate matters: larger early chunks measured
    ~10us SLOWER end-to-end because the higher DMA+ACT+PE activity trips
    the hardware utilization throttle (ACTIVATE durs inflate 1.2x).

Per-core dataflow:
  DMA queue chunk (contiguous fp32) -> DVE cast to bf16
  -> PE 128x128 identity-matmul transposes into PSUM -> DVE copy to SBUF
  -> PE matmul against persistent V2T (bf16) -> logits in PSUM (fp32),
     one 512-col PSUM bank per matmul
  -> ACT exp(10x) in place with fused accumulation (accum_out)
  -> DMA the [128, NT*NG] partial-sum tile out; host sums group partials.
"""

import numpy as np

J, B, F, Q = 2, 256, 128, 65536
NCORES = 8
QC = Q // NCORES          # 8192 queue rows per core
JB = J * B                # 512
INV_T = 10.0
NT = JB // 128            # 4 jb tiles of 128
TEMPERATURE = 0.1

# (row_start, nrows) queue chunks, contiguous per partition; small first
# chunks shorten the pipeline ramp to the first ACT op.
CHUNKS = [(0, 128), (128, 384), (512, 512)] + \
         [(r, 1024) for r in range(1024, QC, 1024)]
assert sum(nr for _, nr in CHUNKS) == QC

# Per-jb-tile scalar-engine exp groups over the 8192 q columns; <= 1536
# cols each (3 PSUM banks; pool bufs=2 + transpose-staging bank = 7 of 8).
# Small leading groups start the ACT engine while the DMA is still ramping.
GROUPS = [
    ('a', 0, 128),
    ('a', 128, 1024),
    ('a', 1024, 2560),
    ('a', 2560, 4096),
    ('a', 4096, 5632),
    ('a', 5632, 7168),
    ('a', 7168, 8192),
]
assert all(c1 - c0 <= 1536 for _, c0, c1 in GROUPS)
assert GROUPS[0][1] == 0 and GROUPS[-1][2] == QC
NG = len(GROUPS)

# Flat (group, tile) op order: ramp groups first, then column order.
SCHED_OPS = [(g, t) for g in range(NG) for t in range(NT)]

_STATE = {}


def _build():
    import concourse.tile as tile
    from concourse import bacc, masks, mybir

    f32 = mybir.dt.float32
    bf16 = mybir.dt.bfloat16
    nc = bacc.Bacc("TRN2", target_bir_lowering=False, debug=False,
                   num_devices=None, enable_partition_id=False)

    vt_d = nc.dram_tensor("V2T", (128, JB), f32, kind="ExternalInput")
    q_d = nc.dram_tensor("queue", (QC, F), f32, kind="ExternalInput")
    # out[p, t*NG + g] = group-g partial of sum_q exp(10 * logit[jb, q]),
    # jb = t*128 + p; the host sums the NG group partials per jb.
    out_d = nc.dram_tensor("out", (128, NT * NG), f32, kind="ExternalOutput")

    # chunk column offsets (transposed q-cols land in chunk order)
    coff = []
    acc_cols = 0
    for _, nr in CHUNKS:
        coff.append(acc_cols)
        acc_cols += nr

    with tile.TileContext(nc) as tc:
        with (
            tc.tile_pool(name="const", bufs=1) as const_pool,
            tc.tile_pool(name="vl", bufs=1) as vl_pool,
            tc.tile_pool(name="qt", bufs=12) as qt_pool,
            tc.tile_pool(name="qtb", bufs=12) as qtb_pool,
            tc.tile_pool(name="qts", bufs=12) as qts_pool,
            tc.tile_pool(name="res", bufs=1) as res_pool,
            tc.tile_pool(name="pslog", bufs=2, space="PSUM") as pslog_pool,
            tc.tile_pool(name="pst", bufs=1, space="PSUM") as pst_pool,
        ):
            identb = const_pool.tile([128, 128], bf16, tag="identb")
            masks.make_identity(nc, identb[:])

            # V2T [f=128, jb=512] comes pre-transposed from the host.
            vt_f = vl_pool.tile([128, JB], f32)
            nc.sync.dma_start(vt_f[:], vt_d.ap())
            v2tb = vl_pool.tile([128, JB], bf16)
            nc.vector.tensor_copy(v2tb[:], vt_f[:])

            # ---- queue chunk pipeline, split in two phases:
            # (a) DMA + bf16 cast, emitted EAGERLY for every chunk up front —
            #     the DMA engines stream at their own pace and the DVE casts
            #     drain behind them;
            # (b) PE transpose + copy-to-SBUF, emitted LAZILY just before the
            #     consuming exp group — a transpose queued in the PE FIFO
            #     before its data arrived would block later, ready matmuls.
            qtbs = []                  # phase-(a) results: bf16 natural tiles
            for r0, nr in CHUNKS:
                sr = nr // 128                         # rows per partition
                qt = qt_pool.tile([128, nr], f32, tag="qt")
                nc.sync.dma_start(
                    qt[:].rearrange("p (s f) -> p s f", f=F),
                    q_d.ap()[r0:r0 + nr, :].rearrange(
                        "(p s) f -> p s f", s=sr))
                qtb = qtb_pool.tile([128, nr], bf16, tag="qtb")
                nc.vector.tensor_copy(qtb[:], qt[:])
                qtbs.append(qtb)

            segs = []                  # (qts_tile, global_col0, n)
            _next_chunk = [0]

            def emit_chunk():
                ci = _next_chunk[0]
                _next_chunk[0] += 1
                _, nr = CHUNKS[ci]
                qtb = qtbs[ci]
                pt = pst_pool.tile([128, nr], bf16, tag="pst")
                for s in range(nr // 128):
                    nc.tensor.transpose(
                        pt[:, s * 128:(s + 1) * 128],
                        qtb[:, s * 128:(s + 1) * 128], identb[:])
                qts = qts_pool.tile([128, nr], bf16, tag="qts")
                nc.vector.tensor_copy(qts[:], pt[:])
                segs.append((qts, coff[ci], nr))

            def ensure_cols(c):
                """Emit transpose pipelines until columns [0, c) are covered."""
                while _next_chunk[0] < len(CHUNKS) and \
                        (not segs or segs[-1][1] + segs[-1][2] < c):
                    emit_chunk()

            PREFETCH = 0               # transposes are just-in-time; casts are eager

            def emit_matmuls(lg, t, c0, c1):
                """Matmuls filling lg[:, 0:c1-c0] with logits for jb tile t,
                global q columns [c0, c1).  Each matmul stays within one
                PSUM bank (512 f32) of lg."""
                for qts, g0, n in segs:
                    o0, o1 = max(c0, g0), min(c1, g0 + n)
                    a = o0
                    while a < o1:
                        b = min(o1, a + 512 - (a - c0) % 512)
                        nc.tensor.matmul(
                            lg[:, a - c0:b - c0],
                            lhsT=v2tb[:, t * 128:(t + 1) * 128],
                            rhs=qts[:, a - g0:b - g0], start=True, stop=True)
                        a = b

            # ---- logits + fused exp/accumulate ----
            # acc[p, t*NG + g] = partial sum for jb tile t, group g
            acc = res_pool.tile([128, NT * NG], f32)
            for gi, t in SCHED_OPS:
                _, c0, c1 = GROUPS[gi]
                ensure_cols(min(c1 + PREFETCH, QC))
                w = c1 - c0
                col = t * NG + gi
                lg = pslog_pool.tile([128, w], f32, tag="pslog")
                emit_matmuls(lg, t, c0, c1)
                nc.scalar.activation(
                    lg[:], lg[:], mybir.ActivationFunctionType.Exp,
                    scale=INV_T, accum_out=acc[:, col:col + 1])

            # ---- finalize: DMA the raw group partials; host sums them ----
            nc.sync.dma_start(out_d.ap(), acc[:])

    nc.compile()
    return nc


def _run(in_maps, trace=False, **kwargs):
    from concourse.bass_utils import run_bass_kernel_spmd
    if "nc" not in _STATE:
        _STATE["nc"] = _build()
    return run_bass_kernel_spmd(_STATE["nc"], in_maps, list(range(NCORES)),
                                trace=trace, **kwargs)


def _make_in_maps(V, L, queue):
    V2T = np.ascontiguousarray(
        np.asarray(V, dtype=np.float32).reshape(JB, F).T)
    qn = np.asarray(queue, dtype=np.float32).reshape(NCORES, QC, F)
    return [{"V2T": V2T, "queue": np.ascontiguousarray(qn[i])}
            for i in range(NCORES)]


def _combine(V, L, outs):
    """outs: list of (128, NT*NG) partial arrays, one per core -> loss."""
    qsum = np.zeros(JB, dtype=np.float64)
    for o in outs:
        per_jb = o.astype(np.float64).reshape(128, NT, NG).sum(-1)
        qsum += per_jb.T.reshape(JB)                 # jb = t*128 + p
    V2 = np.asarray(V, dtype=np.float64).reshape(JB, F)
    Ln = np.asarray(L, dtype=np.float64)
    pos = (V2.reshape(J, B, F) * Ln[None]).sum(-1).reshape(JB) / TEMPERATURE
    batch_sum = np.exp(pos).reshape(J, B).sum(axis=1)  # sum_i exp(pos[j,i])
    denom = np.log(np.repeat(batch_sum, B) + qsum)
    loss = -(pos.sum() - denom.sum()) / B
    return np.array(loss, dtype=np.float32)


def kernel(V, L, queue):
    res = _run(_make_in_maps(V, L, queue))
    return _combine(V, L, [res.results[i]["out"] for i in range(NCORES)])
